# revision 8
# baseline (speedup 1.0000x reference)
"""HardAttention Bass kernel for 8 TRN2 NeuronCores.

reference math (B=32, T=4096, H=256):
  energy[b,t,h] = relu( sum_k cat(hidden,enc)[b,t,k] * attn_w[h,k] + attn_b[h] )
  scores[b,t]   = sum_h energy[b,t,h] * v[h]
  out           = softmax(scores, axis=t)[:, None, :]

Device strategy (data-parallel over B, 4 batches/core):
  * split attn_w into W1 (hidden half) and W2 (encoder half)
  * fold v into W2 and into the per-batch bias q = hidden@W1.T + attn_b
    (valid because v >= 0: relu(x)*v == relu(x*v))
  * per (t-chunk, b): z[h,t] = W2v.T-tiles @ enc_T-tiles (float32r matmuls),
    ACT relu with per-partition bias qv, PE ones-matmul reduces over h
    into a [4, t] PSUM scores tile, ACT exp with fused free-axis accum
  * tail: reciprocal of sum, per-partition scale, contiguous DMA out
Inputs are laid out on the host: enc is transposed to [b, k, t] so k lands
on SBUF partitions with fully contiguous 8KB DMA rows.
"""

from contextlib import ExitStack

import numpy as np

import concourse.bass as bass
import concourse.tile as tile
from concourse import bacc, mybir
from concourse.bass_utils import run_bass_kernel_spmd

B, T, H = 32, 4096, 256
NCORES = 8
BC = B // NCORES            # 4 batches per core
KC = H // 128               # 2 k-chunks
HC = H // 128               # 2 h-chunks
CHUNK = 2048                # t elements per enc DMA tile
NSUB = CHUNK // 512         # matmul free-dim sub-tiles per chunk
NCHUNK = T // CHUNK

F32 = mybir.dt.float32
F32R = mybir.dt.float32r

_CACHE = {}
LAST_RESULTS = None


def _build():
    if "nc" in _CACHE:
        return _CACHE["nc"]

    nc = bacc.Bacc(None, target_bir_lowering=False)
    enc_d = nc.dram_tensor("enc", [BC, KC, 128, T], F32R, kind="ExternalInput")
    w2v_d = nc.dram_tensor("w2v", [KC, HC, 128, 128], F32R, kind="ExternalInput")
    qv_d = nc.dram_tensor("qv", [128, BC * HC], F32, kind="ExternalInput")
    ind_d = nc.dram_tensor("ind", [BC, 128, BC], F32R, kind="ExternalInput")
    out_d = nc.dram_tensor("scores", [BC, T], F32, kind="ExternalOutput")

    AF = mybir.ActivationFunctionType
    ALU = mybir.AluOpType

    with tile.TileContext(nc) as tc, ExitStack() as ctx:
        const = ctx.enter_context(tc.tile_pool(name="const", bufs=1))
        encp = ctx.enter_context(tc.tile_pool(name="encp", bufs=4))
        work = ctx.enter_context(tc.tile_pool(name="work", bufs=4))
        zp = ctx.enter_context(tc.tile_pool(name="zp", bufs=4, space="PSUM"))
        scp = ctx.enter_context(tc.tile_pool(name="scp", bufs=1, space="PSUM"))
        tailp = ctx.enter_context(tc.tile_pool(name="tail", bufs=1))

        w2v_sb = {}
        for kc in range(KC):
            for hc in range(HC):
                t_ = const.tile([128, 128], F32R, tag=f"w2v{kc}{hc}")
                nc.sync.dma_start(t_[:], w2v_d[kc, hc])
                w2v_sb[kc, hc] = t_
        qv_sb = const.tile([128, BC * HC], F32, tag="qv")
        nc.sync.dma_start(qv_sb[:], qv_d[:])
        # indicator lhsT per batch: column b is ones -> ones-matmul lands the
        # h-reduction of batch b on PSUM partition b (accumulated over b)
        ind_sb = []
        for b in range(BC):
            it = const.tile([128, BC], F32R, tag=f"ind{b}")
            nc.sync.dma_start(it[:], ind_d[b])
            ind_sb.append(it)

        exp_sb = tailp.tile([BC, T], F32, tag="exp")
        sums_sb = tailp.tile([BC, NCHUNK], F32, tag="sums")

        for chunk in range(NCHUNK):
            psc = scp.tile([BC, CHUNK], F32, tag="psc")
            for b in range(BC):
                enc_t = []
                for kc in range(KC):
                    et = encp.tile([128, CHUNK], F32R, tag="enc")
                    nc.sync.dma_start(
                        et[:], enc_d[b, kc][:, bass.ts(chunk, CHUNK)]
                    )
                    enc_t.append(et)
                for sub in range(NSUB):
                    relu = []
                    for hc in range(HC):
                        z = zp.tile([128, 512], F32, tag="z")
                        for kc in range(KC):
                            nc.tensor.matmul(
                                z[:],
                                w2v_sb[kc, hc][:],
                                enc_t[kc][:, bass.ts(sub, 512)],
                                start=(kc == 0),
                                stop=(kc == KC - 1),
                            )
                        r = work.tile([128, 512], F32, tag="relu")
                        nc.scalar.activation(
                            r[:], z[:], AF.Relu,
                            bias=qv_sb[:, b * HC + hc : b * HC + hc + 1],
                        )
                        relu.append(r)
                    rs = work.tile([128, 512], F32R, tag="rsum")
                    nc.vector.tensor_add(rs[:], relu[0][:], relu[1][:])
                    nc.tensor.matmul(
                        psc[:, bass.ts(sub, 512)],
                        ind_sb[b][:],
                        rs[:],
                        start=(b == 0),
                        stop=(b == BC - 1),
                    )
            nc.scalar.activation(
                exp_sb[:, bass.ts(chunk, CHUNK)], psc[:], AF.Exp,
                accum_out=sums_sb[:, chunk : chunk + 1],
            )

        stot = tailp.tile([BC, 1], F32, tag="stot")
        nc.vector.tensor_reduce(
            stot[:], sums_sb[:], axis=mybir.AxisListType.X, op=ALU.add
        )
        recip = tailp.tile([BC, 1], F32, tag="recip")
        nc.vector.reciprocal(recip[:], stot[:])
        outs = tailp.tile([BC, T], F32, tag="outs")
        nc.vector.tensor_scalar_mul(outs[:], exp_sb[:], recip[:])
        nc.sync.dma_start(out_d[:], outs[:])

    nc.compile()
    _CACHE["nc"] = nc
    return nc


def _prep_inputs(hidden, encoder_outputs, attn_w, attn_b, v):
    w1 = attn_w[:, :H]
    w2 = attn_w[:, H:]
    qv_full = ((hidden.astype(np.float64) @ w1.T.astype(np.float64)).astype(np.float32)
               + attn_b) * v                       # [B, H]
    qv_full = qv_full.astype(np.float32)
    w2v = (w2 * v[:, None]).astype(np.float32)     # [H(h), H(k)]
    w2v_T = np.ascontiguousarray(w2v.T)            # [k, h]
    w2v_tiles = np.ascontiguousarray(
        w2v_T.reshape(KC, 128, HC, 128).transpose(0, 2, 1, 3)
    )                                              # [kc, hc, kp, hf]

    ind = np.zeros((BC, 128, BC), dtype=np.float32)
    for b in range(BC):
        ind[b, :, b] = 1.0

    in_maps = []
    for c in range(NCORES):
        bs = c * BC
        enc_c = np.ascontiguousarray(
            encoder_outputs[:, bs : bs + BC, :].transpose(1, 2, 0)
        ).reshape(BC, KC, 128, T)
        qv_c = np.ascontiguousarray(
            qv_full[bs : bs + BC].reshape(BC, HC, 128).transpose(2, 0, 1)
        ).reshape(128, BC * HC)
        in_maps.append({"enc": enc_c, "w2v": w2v_tiles, "qv": qv_c, "ind": ind})
    return in_maps


def kernel(hidden, encoder_outputs, attn_w, attn_b, v):
    global LAST_RESULTS
    nc = _build()
    in_maps = _prep_inputs(
        np.asarray(hidden, dtype=np.float32),
        np.asarray(encoder_outputs, dtype=np.float32),
        np.asarray(attn_w, dtype=np.float32),
        np.asarray(attn_b, dtype=np.float32),
        np.asarray(v, dtype=np.float32),
    )
    res = run_bass_kernel_spmd(nc, in_maps, list(range(NCORES)))
    LAST_RESULTS = res
    out = np.empty((B, 1, T), dtype=np.float32)
    for c in range(NCORES):
        out[c * BC : (c + 1) * BC, 0, :] = res.results[c]["scores"]
    return out


# revision 24
# speedup vs baseline: 2.0228x; 2.0228x over previous
"""HardAttention Bass kernel for 8 TRN2 NeuronCores.

reference math (B=32, T=4096, H=256):
  energy[b,t,h] = relu( sum_k cat(hidden,enc)[b,t,k] * attn_w[h,k] + attn_b[h] )
  scores[b,t]   = sum_h energy[b,t,h] * v[h]
  out           = softmax(scores, axis=t)[:, None, :]

Device strategy (data-parallel over B, 4 batches/core):
  * split attn_w into W1 (hidden half) and W2 (encoder half)
  * fold v into W2 and into the per-batch bias q = hidden@W1.T + attn_b
    (valid because v >= 0: relu(x)*v == relu(x*v))
  * per (t-chunk, b): z[h,t] = W2v.T-tiles @ enc_T-tiles (float32r matmuls),
    ACT relu with per-partition bias qv, PE indicator-matmuls reduce over h
    into a [4, t] PSUM scores tile (accumulated across b and h-chunks),
    ACT exp with fused free-axis accum
  * tail: reciprocal of sum, per-partition scale split ACT/DVE, DMA out
Inputs are laid out on the host: enc is transposed to [b, k, t] so k lands
on SBUF partitions with fully contiguous DMA rows.
"""

from contextlib import ExitStack

import numpy as np

import concourse.bass as bass
import concourse.tile as tile
from concourse import bacc, mybir
from concourse.bass_utils import run_bass_kernel_spmd

B, T, H = 32, 4096, 256
NCORES = 8
BC = B // NCORES            # 4 batches per core
KC = H // 128               # 2 k-chunks
HC = H // 128               # 2 h-chunks
# variable t-chunking: small first chunk so the first matmul starts early,
# small last chunks so the end-of-stream pipeline drain is short
CHUNKS = [512, 1024, 1024, 1024, 512]
assert sum(CHUNKS) == T
NCHUNK = len(CHUNKS)

F32 = mybir.dt.float32
F32R = mybir.dt.float32r

_CACHE = {}
LAST_RESULTS = None


def _build():
    if "nc" in _CACHE:
        return _CACHE["nc"]

    nc = bacc.Bacc(None, target_bir_lowering=False)
    enc_d = nc.dram_tensor("enc", [BC, KC, 128, T], F32R, kind="ExternalInput")
    # packed f32r consts: cols [0:512) = w2v lhsT tiles (kc,hc), [512:528) = ind
    wc_d = nc.dram_tensor("wconst", [128, 512 + BC * BC], F32R, kind="ExternalInput")
    qv_d = nc.dram_tensor("qv", [128, BC * HC], F32, kind="ExternalInput")
    out_d = nc.dram_tensor("scores", [BC, T], F32, kind="ExternalOutput")

    AF = mybir.ActivationFunctionType
    ALU = mybir.AluOpType

    with tile.TileContext(nc) as tc, ExitStack() as ctx:
        const = ctx.enter_context(tc.tile_pool(name="const", bufs=1))
        encp = ctx.enter_context(tc.tile_pool(name="encp", bufs=8))
        work = ctx.enter_context(tc.tile_pool(name="work", bufs=4))
        zp = ctx.enter_context(tc.tile_pool(name="zp", bufs=4, space="PSUM"))
        scp = ctx.enter_context(tc.tile_pool(name="scp", bufs=2, space="PSUM"))
        rsp = ctx.enter_context(tc.tile_pool(name="rsp", bufs=14))
        tailp = ctx.enter_context(tc.tile_pool(name="tail", bufs=1))

        wc_sb = const.tile([128, 512 + BC * BC], F32R, tag="wconst")
        nc.scalar.dma_start(wc_sb[:], wc_d[:])
        qv_sb = const.tile([128, BC * HC], F32, tag="qv")
        nc.scalar.dma_start(qv_sb[:], qv_d[:])

        def w2v_ap(kc, hc):
            off = (kc * HC + hc) * 128
            return wc_sb[:, off : off + 128]

        def ind_ap(b):
            off = 512 + b * BC
            return wc_sb[:, off : off + BC]

        exp_sb = tailp.tile([BC, T], F32, tag="exp")
        sums_sb = tailp.tile([BC, NCHUNK], F32, tag="sums")

        # deferred h-reduction matmuls: emitted LAG b-groups behind their
        # producing relu/add so the in-order PE stream never waits on ACT/DVE.
        # exp entries are deferred one step further so they never head-block
        # the in-order ACT queue while their chunk's reductions finish.
        LAG = 1
        queue = []

        def flush_reduce(limit):
            while len(queue) > limit:
                kind, payload = queue.pop(0)
                if kind == "ones":
                    pscq, bq, sq, nsub_q, rsq, fin = payload
                    nc.tensor.matmul(
                        pscq[:, bass.ts(sq, 512)],
                        ind_ap(bq),
                        rsq[:],
                        start=(bq == 0),
                        stop=(bq == BC - 1),
                    )
                    if fin is not None:
                        queue.append(("exp", fin))
                else:
                    cq, pscq, toff, csz = payload
                    nc.scalar.activation(
                        exp_sb[:, toff : toff + csz], pscq[:], AF.Exp,
                        accum_out=sums_sb[:, cq : cq + 1],
                    )

        toff = 0
        gidx = [0]
        for chunk, CHUNK in enumerate(CHUNKS):
            NSUB = CHUNK // 512
            psc = scp.tile([BC, CHUNK], F32, tag="psc")
            for b in range(BC):
                enc_t = []
                for kc in range(KC):
                    et = encp.tile([128, CHUNK], F32R, tag="enc")
                    nc.sync.dma_start(
                        et[:], enc_d[b, kc][:, toff : toff + CHUNK]
                    )
                    enc_t.append(et)
                for sub in range(NSUB):
                    zs = []
                    for hc in range(HC):
                        z = zp.tile([128, 512], F32, tag="z")
                        for kc in range(KC):
                            nc.tensor.matmul(
                                z[:],
                                w2v_ap(kc, hc),
                                enc_t[kc][:, bass.ts(sub, 512)],
                                start=(kc == 0),
                                stop=(kc == KC - 1),
                            )
                        zs.append(z)
                    r0 = work.tile([128, 512], F32, tag="r0")
                    nc.scalar.activation(
                        r0[:], zs[0][:], AF.Relu,
                        bias=qv_sb[:, b * HC : b * HC + 1],
                    )
                    r1 = work.tile([128, 512], F32, tag="r1")
                    # balance relu1 between ACT (3/8) and DVE (5/8)
                    if gidx[0] % 8 in ():
                        nc.scalar.activation(
                            r1[:], zs[1][:], AF.Relu,
                            bias=qv_sb[:, b * HC + 1 : b * HC + 2],
                        )
                    else:
                        nc.vector.tensor_scalar(
                            r1[:], zs[1][:],
                            scalar1=qv_sb[:, b * HC + 1 : b * HC + 2],
                            scalar2=0.0,
                            op0=ALU.add,
                            op1=ALU.max,
                        )
                    gidx[0] += 1
                    rs = rsp.tile([128, 512], F32R, tag="rs")
                    nc.vector.tensor_add(rs[:], r0[:], r1[:])
                    fin = None
                    if b == BC - 1 and sub == NSUB - 1:
                        fin = (chunk, psc, toff, CHUNK)
                    queue.append(("ones", (psc, b, sub, NSUB, rs, fin)))
                flush_reduce(LAG * 2)
            toff += CHUNK
        flush_reduce(0)

        stot = tailp.tile([BC, 1], F32, tag="stot")
        nc.vector.tensor_reduce(
            stot[:], sums_sb[:], axis=mybir.AxisListType.X, op=ALU.add
        )
        recip = tailp.tile([BC, 1], F32, tag="recip")
        nc.vector.reciprocal(recip[:], stot[:])
        outs = tailp.tile([BC, T], F32, tag="outs")
        half = T // 2
        nc.vector.tensor_scalar_mul(
            outs[:, :half], exp_sb[:, :half], recip[:]
        )
        nc.scalar.activation(
            outs[:, half:], exp_sb[:, half:], AF.Copy, scale=recip[:]
        )
        nc.sync.dma_start(out_d[:], outs[:])

    nc.compile()
    _CACHE["nc"] = nc
    return nc


def _prep_inputs(hidden, encoder_outputs, attn_w, attn_b, v):
    w1 = attn_w[:, :H]
    w2 = attn_w[:, H:]
    qv_full = (((hidden @ w1.T) + attn_b) * v).astype(np.float32)   # [B, H]
    w2v = (w2 * v[:, None]).astype(np.float32)     # [H(h), H(k)]
    w2v_T = np.ascontiguousarray(w2v.T)            # [k, h]

    # packed const block: [128, 512+16]
    wconst = np.zeros((128, 512 + BC * BC), dtype=np.float32)
    for kc in range(KC):
        for hc in range(HC):
            off = (kc * HC + hc) * 128
            wconst[:, off : off + 128] = w2v_T[
                kc * 128 : (kc + 1) * 128, hc * 128 : (hc + 1) * 128
            ]
    for b in range(BC):
        wconst[:, 512 + b * BC + b] = 1.0

    in_maps = []
    for c in range(NCORES):
        bs = c * BC
        enc_c = np.ascontiguousarray(
            encoder_outputs[:, bs : bs + BC, :].transpose(1, 2, 0)
        ).reshape(BC, KC, 128, T)
        qv_c = np.ascontiguousarray(
            qv_full[bs : bs + BC].reshape(BC, HC, 128).transpose(2, 0, 1)
        ).reshape(128, BC * HC)
        in_maps.append({"enc": enc_c, "wconst": wconst, "qv": qv_c})
    return in_maps


def kernel(hidden, encoder_outputs, attn_w, attn_b, v):
    global LAST_RESULTS
    nc = _build()
    in_maps = _prep_inputs(
        np.asarray(hidden, dtype=np.float32),
        np.asarray(encoder_outputs, dtype=np.float32),
        np.asarray(attn_w, dtype=np.float32),
        np.asarray(attn_b, dtype=np.float32),
        np.asarray(v, dtype=np.float32),
    )
    res = run_bass_kernel_spmd(nc, in_maps, list(range(NCORES)))
    LAST_RESULTS = res
    out = np.empty((B, 1, T), dtype=np.float32)
    for c in range(NCORES):
        out[c * BC : (c + 1) * BC, 0, :] = res.results[c]["scores"]
    return out


# revision 30
# speedup vs baseline: 189386.6364x; 93624.6331x over previous
"""HardAttention Bass kernel for 8 TRN2 NeuronCores.

reference math (B=32, T=4096, H=256):
  energy[b,t,h] = relu( sum_k cat(hidden,enc)[b,t,k] * attn_w[h,k] + attn_b[h] )
  scores[b,t]   = sum_h energy[b,t,h] * v[h]
  out           = softmax(scores, axis=t)[:, None, :]

Device strategy (data-parallel over B, 4 batches/core):
  * split attn_w into W1 (hidden half) and W2 (encoder half)
  * fold v into W2 and into the per-batch bias q = hidden@W1.T + attn_b
    (valid because v >= 0: relu(x)*v == relu(x*v))
  * per (t-chunk, b): z[h,t] = W2v.T-tiles @ enc_T-tiles (float32r matmuls),
    ACT relu with per-partition bias qv, PE indicator-matmuls reduce over h
    into a [4, t] PSUM scores tile (accumulated across b and h-chunks),
    ACT exp with fused free-axis accum
  * tail: reciprocal of sum, per-partition scale split ACT/DVE, DMA out
Inputs are laid out on the host: enc is transposed to [b, k, t] so k lands
on SBUF partitions with fully contiguous DMA rows.
"""

from contextlib import ExitStack

import numpy as np

import concourse.bass as bass
import concourse.tile as tile
from concourse import bacc, mybir
from concourse.bass_utils import run_bass_kernel_spmd

B, T, H = 32, 4096, 256
NCORES = 8
BC = B // NCORES            # 4 batches per core
KC = H // 128               # 2 k-chunks
HC = H // 128               # 2 h-chunks
# variable t-chunking: small first chunk so the first matmul starts early,
# small last chunks so the end-of-stream pipeline drain is short
CHUNKS = [512, 1024, 1024, 1024, 512]
assert sum(CHUNKS) == T
NCHUNK = len(CHUNKS)

F32 = mybir.dt.float32
F32R = mybir.dt.float32r

_CACHE = {}
LAST_RESULTS = None


def _build():
    if "nc" in _CACHE:
        return _CACHE["nc"]

    nc = bacc.Bacc(None, target_bir_lowering=False)
    enc_d = nc.dram_tensor("enc", [BC, KC, 128, T], F32R, kind="ExternalInput")
    # packed f32r consts: cols [0:512) = w2v lhsT tiles (kc,hc), [512:528) = ind
    wc_d = nc.dram_tensor("wconst", [128, 512 + BC * BC], F32R, kind="ExternalInput")
    qv_d = nc.dram_tensor("qv", [128, BC * HC], F32, kind="ExternalInput")
    out_d = nc.dram_tensor("scores", [BC, T], F32, kind="ExternalOutput")

    AF = mybir.ActivationFunctionType
    ALU = mybir.AluOpType

    with tile.TileContext(nc) as tc, ExitStack() as ctx:
        const = ctx.enter_context(tc.tile_pool(name="const", bufs=1))
        encp = ctx.enter_context(tc.tile_pool(name="encp", bufs=12))
        work = ctx.enter_context(tc.tile_pool(name="work", bufs=4))
        zp = ctx.enter_context(tc.tile_pool(name="zp", bufs=4, space="PSUM"))
        scp = ctx.enter_context(tc.tile_pool(name="scp", bufs=2, space="PSUM"))
        rsp = ctx.enter_context(tc.tile_pool(name="rsp", bufs=8))
        tailp = ctx.enter_context(tc.tile_pool(name="tail", bufs=1))

        wc_sb = const.tile([128, 512 + BC * BC], F32R, tag="wconst")
        nc.scalar.dma_start(wc_sb[:], wc_d[:])
        qv_sb = const.tile([128, BC * HC], F32, tag="qv")
        nc.scalar.dma_start(qv_sb[:], qv_d[:])

        def w2v_ap(kc, hc):
            off = (kc * HC + hc) * 128
            return wc_sb[:, off : off + 128]

        def ind_ap(b):
            off = 512 + b * BC
            return wc_sb[:, off : off + BC]

        exp_sb = tailp.tile([BC, T], F32, tag="exp")
        sums_sb = tailp.tile([BC, NCHUNK], F32, tag="sums")

        # deferred h-reduction matmuls: emitted LAG b-groups behind their
        # producing relu/add so the in-order PE stream never waits on ACT/DVE.
        # exp entries are deferred one step further so they never head-block
        # the in-order ACT queue while their chunk's reductions finish.
        LAG = 3
        queue = []

        def flush_reduce(limit):
            while len(queue) > limit:
                kind, payload = queue.pop(0)
                if kind == "ones":
                    pscq, bq, sq, r0q, r1q, fin = payload
                    nc.tensor.matmul(
                        pscq[:, bass.ts(sq, 512)],
                        ind_ap(bq),
                        r0q[:],
                        start=(bq == 0),
                        stop=False,
                    )
                    nc.tensor.matmul(
                        pscq[:, bass.ts(sq, 512)],
                        ind_ap(bq),
                        r1q[:],
                        start=False,
                        stop=(bq == BC - 1),
                    )
                    if fin is not None:
                        queue.append(("exp", fin))
                else:
                    cq, pscq, toff, csz = payload
                    nc.scalar.activation(
                        exp_sb[:, toff : toff + csz], pscq[:], AF.Exp,
                        accum_out=sums_sb[:, cq : cq + 1],
                    )

        toff = 0
        gidx = [0]
        for chunk, CHUNK in enumerate(CHUNKS):
            NSUB = CHUNK // 512
            psc = scp.tile([BC, CHUNK], F32, tag="psc")
            for b in range(BC):
                enc_t = []
                for kc in range(KC):
                    et = encp.tile([128, CHUNK], F32R, tag="enc")
                    nc.sync.dma_start(
                        et[:], enc_d[b, kc][:, toff : toff + CHUNK]
                    )
                    enc_t.append(et)
                for sub in range(NSUB):
                    zs = []
                    for hc in range(HC):
                        z = zp.tile([128, 512], F32, tag="z")
                        for kc in range(KC):
                            nc.tensor.matmul(
                                z[:],
                                w2v_ap(kc, hc),
                                enc_t[kc][:, bass.ts(sub, 512)],
                                start=(kc == 0),
                                stop=(kc == KC - 1),
                            )
                        zs.append(z)
                    r0 = rsp.tile([128, 512], F32R, tag="r0")
                    nc.scalar.activation(
                        r0[:], zs[0][:], AF.Relu,
                        bias=qv_sb[:, b * HC : b * HC + 1],
                    )
                    r1 = rsp.tile([128, 512], F32R, tag="r1")
                    # balance relu1 between ACT (3/8) and DVE (5/8)
                    if gidx[0] % 8 in ():
                        nc.scalar.activation(
                            r1[:], zs[1][:], AF.Relu,
                            bias=qv_sb[:, b * HC + 1 : b * HC + 2],
                        )
                    else:
                        nc.vector.tensor_scalar(
                            r1[:], zs[1][:],
                            scalar1=qv_sb[:, b * HC + 1 : b * HC + 2],
                            scalar2=0.0,
                            op0=ALU.add,
                            op1=ALU.max,
                        )
                    gidx[0] += 1
                    fin = None
                    if b == BC - 1 and sub == NSUB - 1:
                        fin = (chunk, psc, toff, CHUNK)
                    queue.append(("ones", (psc, b, sub, r0, r1, fin)))
                flush_reduce(LAG * 2)
            toff += CHUNK
        flush_reduce(0)

        stot = tailp.tile([BC, 1], F32, tag="stot")
        nc.vector.tensor_reduce(
            stot[:], sums_sb[:], axis=mybir.AxisListType.X, op=ALU.add
        )
        recip = tailp.tile([BC, 1], F32, tag="recip")
        nc.vector.reciprocal(recip[:], stot[:])
        outs = tailp.tile([BC, T], F32, tag="outs")
        half = T // 2
        nc.vector.tensor_scalar_mul(
            outs[:, :half], exp_sb[:, :half], recip[:]
        )
        nc.scalar.activation(
            outs[:, half:], exp_sb[:, half:], AF.Copy, scale=recip[:]
        )
        nc.sync.dma_start(out_d[:], outs[:])

    nc.compile()
    _CACHE["nc"] = nc
    return nc


def _prep_inputs(hidden, encoder_outputs, attn_w, attn_b, v):
    w1 = attn_w[:, :H]
    w2 = attn_w[:, H:]
    qv_full = (((hidden @ w1.T) + attn_b) * v).astype(np.float32)   # [B, H]
    w2v = (w2 * v[:, None]).astype(np.float32)     # [H(h), H(k)]
    w2v_T = np.ascontiguousarray(w2v.T)            # [k, h]

    # packed const block: [128, 512+16]
    wconst = np.zeros((128, 512 + BC * BC), dtype=np.float32)
    for kc in range(KC):
        for hc in range(HC):
            off = (kc * HC + hc) * 128
            wconst[:, off : off + 128] = w2v_T[
                kc * 128 : (kc + 1) * 128, hc * 128 : (hc + 1) * 128
            ]
    for b in range(BC):
        wconst[:, 512 + b * BC + b] = 1.0

    in_maps = []
    for c in range(NCORES):
        bs = c * BC
        enc_c = np.ascontiguousarray(
            encoder_outputs[:, bs : bs + BC, :].transpose(1, 2, 0)
        ).reshape(BC, KC, 128, T)
        qv_c = np.ascontiguousarray(
            qv_full[bs : bs + BC].reshape(BC, HC, 128).transpose(2, 0, 1)
        ).reshape(128, BC * HC)
        in_maps.append({"enc": enc_c, "wconst": wconst, "qv": qv_c})
    return in_maps


def kernel(hidden, encoder_outputs, attn_w, attn_b, v):
    global LAST_RESULTS
    nc = _build()
    in_maps = _prep_inputs(
        np.asarray(hidden, dtype=np.float32),
        np.asarray(encoder_outputs, dtype=np.float32),
        np.asarray(attn_w, dtype=np.float32),
        np.asarray(attn_b, dtype=np.float32),
        np.asarray(v, dtype=np.float32),
    )
    res = run_bass_kernel_spmd(nc, in_maps, list(range(NCORES)))
    LAST_RESULTS = res
    out = np.empty((B, 1, T), dtype=np.float32)
    for c in range(NCORES):
        out[c * BC : (c + 1) * BC, 0, :] = res.results[c]["scores"]
    return out
